# revision 8
# baseline (speedup 1.0000x reference)
"""Trainium2 Bass kernel for the Neural-CDE-style cell (nn_JaCDE_88167088653055).

Math (per batch row b):
    x    = spline(coeffs, t)   xdot = spline(dcoeffs, t)
    l1   = x @ wx.T + h @ wh.T + b0
    relu = relu(l1);  drelu = sigmoid(l1)
    lout = relu @ wout.T + b1; th = tanh(lout); dth = 1 - th^2
    J(v) = dth * ((drelu * v) @ wout.T)        # action of the Jacobian factor
    jx   = J(xdot @ wx.T); jxh = J(jx @ wh.T); jxhh = J(jxh @ wh.T)
    out  = jx + jxh + jxhh

Device-side structure (per core, batch-feature-major [H, batch] tiles):
  * spline folds into the wx matmul: x @ wx.T == csel_flat @ (wx (x) powers).T,
    so the contraction is K=256 and the spline costs no device passes.
  * dth = 1-tanh^2 computed as ACT Tanh then ACT Square (thq); the m-matmuls
    use a negated stationary -wout so every dth-multiply is a single
    scalar_tensor_tensor (thq-1)*M = dth*(wout@p).
  * m1+m2+m3 accumulate in ONE PSUM bank via PE start/stop flags; jx = dth*M1,
    t2 = dth*M2 (= jx+jxh), out = dth*M3 directly.  jxh is never formed:
    the g bank holds -Wh@jx, and accumulating +Wh@t2 onto it yields
    Wh@jxh directly (the sign flip is absorbed by using +wout for m2).
  * the two 512-column chunks are software-pipelined stage-by-stage so every
    engine queue alternates A/B work (in-order queues never head-of-line block
    a ready chunk behind a stalled one).
  * warmup: a dummy Sigmoid first (loads the single act table covering
    Relu+Sigmoid during the input DMA) and dummy matmuls on a zeroed tile to
    ramp the PE clock out of its low p-state before real work arrives.

Sharding: pure data parallel - batch 8192 split as 1024 rows per core across
8 cores; small weights replicated; no cross-core communication.
"""

import numpy as np
import ml_dtypes

import concourse.bass as bass
import concourse.mybir as mybir
import concourse.tile as tile
from concourse import bacc, bass_utils

N_CORES = 8
B = 8192
NOBS = 16
CIN = 64
H = 128
K4 = CIN * 4            # 256: folded (channel, power) contraction dim
BS = B // N_CORES       # 1024 batch rows per core
CHUNK = 512             # batch columns per pipeline step (one PSUM bank)
NCH = BS // CHUNK       # 2
F32 = mybir.dt.float32
BF16 = mybir.dt.bfloat16
NPBF16 = ml_dtypes.bfloat16

DUMMY_MM = 10           # PE-clock warmup matmuls (N=256 each) on zeroed data

_NC_CACHE = {}


def _build_nc():
    AF = mybir.ActivationFunctionType
    OP = mybir.AluOpType

    nc = bacc.Bacc("TRN2", target_bir_lowering=False, debug=False,
                   enable_asserts=False, num_devices=N_CORES)

    # inputs packed per chunk as [c0|c1|d0|d1|h], 5*CHUNK bf16 cols per chunk
    xint = nc.dram_tensor("xint", [128, NCH * 5 * CHUNK], BF16,
                          kind="ExternalInput")
    # weights packed [wxp0|wxp1|wht|woutt|-woutt|-wht]
    wtst = nc.dram_tensor("wtst", [128, 6 * H], BF16, kind="ExternalInput")
    bst = nc.dram_tensor("bst", [H, 2], F32, kind="ExternalInput")
    outt = nc.dram_tensor("outt", [H, BS], BF16, kind="ExternalOutput")

    def mm(out_ap, lhsT, rhs, start=True, stop=True):
        nc.tensor.matmul(out_ap, lhsT, rhs, start=start, stop=stop,
                         skip_group_check=True)

    with tile.TileContext(nc) as tc:
        with tc.tile_pool(name="w", bufs=1) as wp, \
             tc.tile_pool(name="io", bufs=2) as io, \
             tc.tile_pool(name="tmp", bufs=2) as tmp, \
             tc.tile_pool(name="ps", bufs=1, space="PSUM") as ps:

            # ---- warmup + weight/bias/input loads (t=0) ----
            dmy = wp.tile([128, CHUNK], BF16, tag="dmy")
            nc.gpsimd.memset(dmy[:], 0.0)

            wts = wp.tile([128, 6 * H], BF16, tag="wts")
            nc.sync.dma_start(wts[:], wtst[:])
            bs = wp.tile([H, 2], F32, tag="bs")
            nc.sync.dma_start(bs[:], bst[:])

            xin = []
            dmaq = [nc.sync, nc.scalar, nc.gpsimd]
            for ch in range(NCH):
                xt = io.tile([128, 5 * CHUNK], BF16, tag="xin")
                base = ch * 5 * CHUNK
                for blk in range(5):  # [c0|c1|h... order c0,c1,h,d0,d1]
                    lo_, hi_ = blk * CHUNK, (blk + 1) * CHUNK
                    dmaq[(ch * 5 + blk) % 3].dma_start(
                        xt[:, lo_:hi_], xint[:, base + lo_:base + hi_])
                xin.append(xt)

            # act-table warmup: Sigmoid first -> single table set load that
            # also covers Relu, overlapped with the input DMA
            wrm = tmp.tile([128, 1], F32, tag="wrm")
            nc.scalar.activation(wrm[:], dmy[:, 0:1], AF.Sigmoid)

            wxp0 = wts[:, 0 * H:1 * H]
            wxp1 = wts[:, 1 * H:2 * H]
            wht = wts[:, 2 * H:3 * H]
            wot = wts[:, 3 * H:4 * H]
            won = wts[:, 4 * H:5 * H]   # -wout
            whn = wts[:, 5 * H:6 * H]   # -wh
            b0ap = bs[:, 0:1]
            b1ap = bs[:, 1:2]

            # PE-clock warmup: dummy matmuls on zeroed data into the lo bank
            lo_warm = ps.tile([H, CHUNK], F32, tag="lo")
            for _ in range(DUMMY_MM):
                mm(lo_warm[:, 0:256], dmy[:, 0:H], dmy[:, 0:256])

            # ---- software-pipelined chunk stages ----
            l1 = [None] * NCH
            u = [None] * NCH
            relu = [None] * NCH
            drelu = [None] * NCH
            lo = [None] * NCH
            q = [None] * NCH
            m = [None] * NCH
            g = [None] * NCH
            jx = [None] * NCH

            # F: front-end matmuls + relu/drelu
            for ch in range(NCH):
                xt = xin[ch]
                c0 = xt[:, 0 * CHUNK:1 * CHUNK]
                c1 = xt[:, 1 * CHUNK:2 * CHUNK]
                ht = xt[:, 2 * CHUNK:3 * CHUNK]
                d0 = xt[:, 3 * CHUNK:4 * CHUNK]
                d1 = xt[:, 4 * CHUNK:5 * CHUNK]
                l1[ch] = ps.tile([H, CHUNK], F32, tag="l1", bufs=2, name="l1")
                u[ch] = ps.tile([H, CHUNK], F32, tag="u", bufs=2, name="u")
                mm(l1[ch][:], wxp0, c0, start=True, stop=False)
                mm(l1[ch][:], wxp1, c1, start=False, stop=False)
                mm(l1[ch][:], wht, ht, start=False, stop=True)
                mm(u[ch][:], wxp0, d0, start=True, stop=False)
                mm(u[ch][:], wxp1, d1, start=False, stop=True)
                relu[ch] = tmp.tile([H, CHUNK], BF16, tag="relu", name="relu")
                nc.scalar.activation(relu[ch][:], l1[ch][:], AF.Relu,
                                     bias=b0ap)
                drelu[ch] = tmp.tile([H, CHUNK], F32, tag="drelu", name="drelu")
                nc.scalar.activation(drelu[ch][:], l1[ch][:], AF.Sigmoid,
                                     bias=b0ap)

            # C1: lout, s, q, p1, m1
            for ch in range(NCH):
                lo[ch] = ps.tile([H, CHUNK], F32, tag="lo", name="lo")
                mm(lo[ch][:], wot, relu[ch][:])
                th = tmp.tile([H, CHUNK], F32, tag="th")
                nc.scalar.activation(th[:], lo[ch][:], AF.Tanh, bias=b1ap)
                q[ch] = tmp.tile([H, CHUNK], F32, tag="q", name="q")
                nc.scalar.activation(q[ch][:], th[:], AF.Square)
                p1 = tmp.tile([H, CHUNK], BF16, tag="p1")
                nc.vector.tensor_mul(p1[:], drelu[ch][:], u[ch][:])
                m[ch] = ps.tile([H, CHUNK], F32, tag="m", bufs=2, name="m")
                mm(m[ch][:], won, p1[:], start=True, stop=False)

            # C2: jx = dth*M1, g1, p2, m2  (dth = -4q; 4 folded into wo4)
            for ch in range(NCH):
                jx[ch] = tmp.tile([H, CHUNK], BF16, tag="jx", name="jx")
                nc.vector.scalar_tensor_tensor(jx[ch][:], q[ch][:], 1.0,
                                               m[ch][:], OP.subtract, OP.mult)
                g[ch] = ps.tile([H, CHUNK], F32, tag="g", name="g")
                mm(g[ch][:], whn, jx[ch][:], start=True, stop=False)
                p2 = tmp.tile([H, CHUNK], BF16, tag="p2")
                nc.vector.tensor_mul(p2[:], drelu[ch][:], g[ch][:])
                mm(m[ch][:], wot, p2[:], start=False, stop=False)

            # C3: t2 = dth*M2 = jx+jxh, jxh = t2-jx, g2, p3, m3
            for ch in range(NCH):
                t2 = tmp.tile([H, CHUNK], BF16, tag="t2")
                nc.vector.scalar_tensor_tensor(t2[:], q[ch][:], 1.0,
                                               m[ch][:], OP.subtract, OP.mult)
                jxh = tmp.tile([H, CHUNK], BF16, tag="jxh")
                nc.gpsimd.tensor_sub(jxh[:], t2[:], jx[ch][:])
                g2 = ps.tile([H, CHUNK], F32, tag="g")
                mm(g2[:], wht, jxh[:])
                p3 = tmp.tile([H, CHUNK], BF16, tag="p3")
                nc.vector.tensor_mul(p3[:], drelu[ch][:], g2[:])
                mm(m[ch][:], won, p3[:], start=False, stop=True)

            # OUT: out = dth*M3 = jx+jxh+jxhh
            for ch in range(NCH):
                outs = tmp.tile([H, CHUNK], BF16, tag="outs")
                nc.vector.scalar_tensor_tensor(outs[:], q[ch][:], 1.0,
                                               m[ch][:], OP.subtract, OP.mult)
                cs = bass.ts(ch, CHUNK)
                nc.gpsimd.dma_start(outt[:, cs], outs[:])

    nc.compile()
    return nc


def _get_nc():
    if "nc" not in _NC_CACHE:
        _NC_CACHE["nc"] = _build_nc()
    return _NC_CACHE["nc"]


def _prep_in_maps(t, h, coeffs, dcoeffs, tobs, wx, wh, wout, b0, b1):
    t = np.asarray(t, np.float32)
    h = np.asarray(h, np.float32)
    coeffs = np.asarray(coeffs, np.float32)
    dcoeffs = np.asarray(dcoeffs, np.float32)
    tobs = np.asarray(tobs, np.float32)
    wx = np.asarray(wx, np.float32)
    wh = np.asarray(wh, np.float32)
    wout = np.asarray(wout, np.float32)
    b0 = np.asarray(b0, np.float32)
    b1 = np.asarray(b1, np.float32)

    ts = t[0]
    idx = int(np.clip(np.searchsorted(tobs, ts, side="right") - 1, 0, NOBS - 2))
    dtv = np.float32(ts - tobs[idx])
    powers = dtv ** np.arange(4, dtype=np.float32)            # [4]
    wxp = (wx[:, :, None] * powers[None, None, :]).reshape(H, K4)

    wxpt = wxp.T                                              # [256, 128]
    # packed stationaries: [wxp0|wxp1|wht|woutt|4*woutt], bf16
    wts = np.concatenate(
        [wxpt[0:128], wxpt[128:256], wh.T, wout.T, -wout.T, -wh.T],
        axis=1).astype(NPBF16)
    bst = np.stack([b0, b1], axis=1).astype(np.float32)        # [H, 2]

    csel = coeffs[:, idx].reshape(B, K4)                      # [B, 256]
    dsel = dcoeffs[:, idx].reshape(B, K4)
    cselT = csel.T.astype(NPBF16)                             # [256, B]
    dselT = dsel.T.astype(NPBF16)
    hT = h.T.astype(NPBF16)                                   # [128, B]

    in_maps = []
    for c in range(N_CORES):
        sl = slice(c * BS, (c + 1) * BS)
        blocks = []
        for ch in range(NCH):
            s2 = slice(c * BS + ch * CHUNK, c * BS + (ch + 1) * CHUNK)
            blocks += [cselT[0:128, s2], cselT[128:256, s2], hT[:, s2],
                       dselT[0:128, s2], dselT[128:256, s2]]
        xint = np.ascontiguousarray(np.concatenate(blocks, axis=1))
        in_maps.append({"xint": xint, "wtst": wts, "bst": bst})
    return in_maps


def kernel(**inputs) -> np.ndarray:
    in_maps = _prep_in_maps(**inputs)
    nc = _get_nc()
    res = bass_utils.run_bass_kernel_spmd(nc, in_maps,
                                          core_ids=list(range(N_CORES)))
    out = np.empty((B, H), np.float32)
    for c in range(N_CORES):
        out[c * BS:(c + 1) * BS] = res.results[c]["outt"].astype(np.float32).T
    return out


# revision 10
# speedup vs baseline: 1.0929x; 1.0929x over previous
"""Trainium2 Bass kernel for the Neural-CDE-style cell (nn_JaCDE_88167088653055).

Math (per batch row b):
    x    = spline(coeffs, t)   xdot = spline(dcoeffs, t)
    l1   = x @ wx.T + h @ wh.T + b0
    relu = relu(l1);  drelu = sigmoid(l1)
    lout = relu @ wout.T + b1; th = tanh(lout); dth = 1 - th^2
    J(v) = dth * ((drelu * v) @ wout.T)        # action of the Jacobian factor
    jx   = J(xdot @ wx.T); jxh = J(jx @ wh.T); jxhh = J(jxh @ wh.T)
    out  = jx + jxh + jxhh

Device-side structure (per core, batch-feature-major [H, batch] tiles):
  * spline folds into the wx matmul: x @ wx.T == csel_flat @ (wx (x) powers).T,
    so the contraction is K=256 and the spline costs no device passes.
  * dth = 1-tanh^2 computed as ACT Tanh then ACT Square (thq); the m-matmuls
    use a negated stationary -wout so every dth-multiply is a single
    scalar_tensor_tensor (thq-1)*M = dth*(wout@p).
  * m1+m2+m3 accumulate in ONE PSUM bank via PE start/stop flags; jx = dth*M1,
    t2 = dth*M2 (= jx+jxh), out = dth*M3 directly.  jxh is never formed:
    the g bank holds -Wh@jx, and accumulating +Wh@t2 onto it yields
    Wh@jxh directly (the sign flip is absorbed by using +wout for m2).
  * the two 512-column chunks are software-pipelined stage-by-stage so every
    engine queue alternates A/B work (in-order queues never head-of-line block
    a ready chunk behind a stalled one).
  * warmup: a dummy Sigmoid first (loads the single act table covering
    Relu+Sigmoid during the input DMA) and dummy matmuls on a zeroed tile to
    ramp the PE clock out of its low p-state before real work arrives.

Sharding: pure data parallel - batch 8192 split as 1024 rows per core across
8 cores; small weights replicated; no cross-core communication.
"""

import numpy as np
import ml_dtypes

import concourse.bass as bass
import concourse.mybir as mybir
import concourse.tile as tile
from concourse import bacc, bass_utils

N_CORES = 8
B = 8192
NOBS = 16
CIN = 64
H = 128
K4 = CIN * 4            # 256: folded (channel, power) contraction dim
BS = B // N_CORES       # 1024 batch rows per core
CHUNK = 512             # batch columns per pipeline step (one PSUM bank)
NCH = BS // CHUNK       # 2
F32 = mybir.dt.float32
BF16 = mybir.dt.bfloat16
NPBF16 = ml_dtypes.bfloat16

DUMMY_MM = 9            # PE-clock warmup matmuls (N=256 each) on zeroed data

_NC_CACHE = {}


def _build_nc():
    AF = mybir.ActivationFunctionType
    OP = mybir.AluOpType

    nc = bacc.Bacc("TRN2", target_bir_lowering=False, debug=False,
                   enable_asserts=False, num_devices=N_CORES)

    # inputs packed per chunk as [c0|c1|d0|d1|h], 5*CHUNK bf16 cols per chunk
    xint = nc.dram_tensor("xint", [128, NCH * 5 * CHUNK], BF16,
                          kind="ExternalInput")
    # weights packed [wxp0|wxp1|wht|woutt|-woutt|-wht]
    wtst = nc.dram_tensor("wtst", [128, 6 * H], BF16, kind="ExternalInput")
    bst = nc.dram_tensor("bst", [H, 2], F32, kind="ExternalInput")
    outt = nc.dram_tensor("outt", [H, BS], BF16, kind="ExternalOutput")

    def mm(out_ap, lhsT, rhs, start=True, stop=True):
        nc.tensor.matmul(out_ap, lhsT, rhs, start=start, stop=stop,
                         skip_group_check=True)

    with tile.TileContext(nc) as tc:
        with tc.tile_pool(name="sb", bufs=2) as sb, \
             tc.tile_pool(name="ps", bufs=1, space="PSUM") as ps:
            wp = io = tmp = sb

            # ---- warmup + weight/bias/input loads (t=0) ----
            dmy = wp.tile([128, CHUNK], BF16, tag="dmy")
            nc.gpsimd.memset(dmy[:], 0.0)

            wts = wp.tile([128, 6 * H], BF16, tag="wts")
            nc.sync.dma_start(wts[:], wtst[:])
            bs = wp.tile([H, 2], F32, tag="bs")

            xin = []
            for ch in range(NCH):
                xt = io.tile([128, 5 * CHUNK], BF16, tag="xin")
                xin.append(xt)

            def ld(ch, blk, eng):
                lo_, hi_ = blk * CHUNK, (blk + 1) * CHUNK
                base = ch * 5 * CHUNK
                eng.dma_start(xin[ch][:, lo_:hi_], xint[:, base + lo_:base + hi_])

            # priority order: chunk A's l1 operands + weights stream first
            ld(0, 0, nc.sync)    # c0A
            ld(0, 1, nc.scalar)  # c1A
            ld(0, 2, nc.gpsimd)  # hA
            ld(0, 3, nc.sync)    # d0A
            ld(0, 4, nc.scalar)  # d1A
            nc.gpsimd.dma_start(bs[:], bst[:])
            ld(1, 0, nc.sync)    # c0B
            ld(1, 1, nc.scalar)  # c1B
            ld(1, 2, nc.gpsimd)  # hB
            ld(1, 3, nc.sync)    # d0B
            ld(1, 4, nc.scalar)  # d1B

            # act-table warmup: Sigmoid first -> single table set load that
            # also covers Relu, overlapped with the input DMA
            wrm = tmp.tile([128, 1], F32, tag="wrm")
            nc.scalar.activation(wrm[:], dmy[:, 0:1], AF.Sigmoid)

            wxp0 = wts[:, 0 * H:1 * H]
            wxp1 = wts[:, 1 * H:2 * H]
            wht = wts[:, 2 * H:3 * H]
            wot = wts[:, 3 * H:4 * H]
            won = wts[:, 4 * H:5 * H]   # -wout
            whn = wts[:, 5 * H:6 * H]   # -wh
            b0ap = bs[:, 0:1]
            b1ap = bs[:, 1:2]

            # PE-clock warmup: dummy matmuls on zeroed data into the lo bank
            lo_warm = ps.tile([H, CHUNK], F32, tag="lo")
            for _ in range(DUMMY_MM):
                mm(lo_warm[:, 0:256], dmy[:, 0:H], dmy[:, 0:256])

            # ---- software-pipelined chunk stages ----
            l1 = [None] * NCH
            u = [None] * NCH
            relu = [None] * NCH
            drelu = [None] * NCH
            lo = [None] * NCH
            q = [None] * NCH
            m = [None] * NCH
            g = [None] * NCH
            jx = [None] * NCH

            # F: front-end matmuls + relu/drelu
            for ch in range(NCH):
                xt = xin[ch]
                c0 = xt[:, 0 * CHUNK:1 * CHUNK]
                c1 = xt[:, 1 * CHUNK:2 * CHUNK]
                ht = xt[:, 2 * CHUNK:3 * CHUNK]
                d0 = xt[:, 3 * CHUNK:4 * CHUNK]
                d1 = xt[:, 4 * CHUNK:5 * CHUNK]
                l1[ch] = ps.tile([H, CHUNK], F32, tag="l1", bufs=1, name="l1")
                u[ch] = ps.tile([H, CHUNK], F32, tag="u", bufs=2, name="u")
                mm(l1[ch][:], wxp0, c0, start=True, stop=False)
                mm(l1[ch][:], wxp1, c1, start=False, stop=False)
                mm(l1[ch][:], wht, ht, start=False, stop=True)
                mm(u[ch][:], wxp0, d0, start=True, stop=False)
                mm(u[ch][:], wxp1, d1, start=False, stop=True)
                relu[ch] = tmp.tile([H, CHUNK], BF16, tag="relu", name="relu")
                nc.scalar.activation(relu[ch][:], l1[ch][:], AF.Relu,
                                     bias=b0ap)
                drelu[ch] = tmp.tile([H, CHUNK], F32, tag="drelu", name="drelu")
                nc.scalar.activation(drelu[ch][:], l1[ch][:], AF.Sigmoid,
                                     bias=b0ap)

            # C1: lout, s, q, p1, m1
            for ch in range(NCH):
                lo[ch] = ps.tile([H, CHUNK], F32, tag="lo", name="lo")
                mm(lo[ch][:], wot, relu[ch][:])
                th = tmp.tile([H, CHUNK], F32, tag="th")
                nc.scalar.activation(th[:], lo[ch][:], AF.Tanh, bias=b1ap)
                q[ch] = tmp.tile([H, CHUNK], F32, tag="q", name="q")
                nc.scalar.activation(q[ch][:], th[:], AF.Square)
                p1 = tmp.tile([H, CHUNK], BF16, tag="p1")
                nc.vector.tensor_mul(p1[:], drelu[ch][:], u[ch][:])
                m[ch] = ps.tile([H, CHUNK], F32, tag="m", bufs=2, name="m")
                mm(m[ch][:], won, p1[:], start=True, stop=False)

            # C2: jx = dth*M1, g1, p2, m2  (dth = -4q; 4 folded into wo4)
            for ch in range(NCH):
                jx[ch] = tmp.tile([H, CHUNK], BF16, tag="jx", name="jx")
                nc.vector.scalar_tensor_tensor(jx[ch][:], q[ch][:], 1.0,
                                               m[ch][:], OP.subtract, OP.mult)
                g[ch] = ps.tile([H, CHUNK], F32, tag="g", bufs=2, name="g")
                mm(g[ch][:], whn, jx[ch][:], start=True, stop=False)
                p2 = tmp.tile([H, CHUNK], BF16, tag="p2")
                nc.vector.tensor_mul(p2[:], drelu[ch][:], g[ch][:])
                mm(m[ch][:], wot, p2[:], start=False, stop=False)

            # C3: t2 = dth*M2 = jx+jxh; accumulate Wh@t2 onto g (= -Wh@jx)
            # so the bank becomes Wh@jxh with no explicit subtract; then
            # p3, m3, and the final out = dth*M3 + its DMA per chunk.
            for ch in range(NCH):
                t2 = tmp.tile([H, CHUNK], BF16, tag="t2")
                nc.vector.scalar_tensor_tensor(t2[:], q[ch][:], 1.0,
                                               m[ch][:], OP.subtract, OP.mult)
                mm(g[ch][:], wht, t2[:], start=False, stop=True)  # = Wh@jxh
                p3 = tmp.tile([H, CHUNK], BF16, tag="p3")
                nc.vector.tensor_mul(p3[:], drelu[ch][:], g[ch][:])
                mm(m[ch][:], won, p3[:], start=False, stop=True)
                outs = tmp.tile([H, CHUNK], BF16, tag="outs")
                nc.vector.scalar_tensor_tensor(outs[:], q[ch][:], 1.0,
                                               m[ch][:], OP.subtract, OP.mult)
                cs = bass.ts(ch, CHUNK)
                nc.gpsimd.dma_start(outt[:, cs], outs[:])

    nc.compile()
    return nc


def _get_nc():
    if "nc" not in _NC_CACHE:
        _NC_CACHE["nc"] = _build_nc()
    return _NC_CACHE["nc"]


def _prep_in_maps(t, h, coeffs, dcoeffs, tobs, wx, wh, wout, b0, b1):
    t = np.asarray(t, np.float32)
    h = np.asarray(h, np.float32)
    coeffs = np.asarray(coeffs, np.float32)
    dcoeffs = np.asarray(dcoeffs, np.float32)
    tobs = np.asarray(tobs, np.float32)
    wx = np.asarray(wx, np.float32)
    wh = np.asarray(wh, np.float32)
    wout = np.asarray(wout, np.float32)
    b0 = np.asarray(b0, np.float32)
    b1 = np.asarray(b1, np.float32)

    ts = t[0]
    idx = int(np.clip(np.searchsorted(tobs, ts, side="right") - 1, 0, NOBS - 2))
    dtv = np.float32(ts - tobs[idx])
    powers = dtv ** np.arange(4, dtype=np.float32)            # [4]
    wxp = (wx[:, :, None] * powers[None, None, :]).reshape(H, K4)

    wxpt = wxp.T                                              # [256, 128]
    # packed stationaries: [wxp0|wxp1|wht|woutt|4*woutt], bf16
    wts = np.concatenate(
        [wxpt[0:128], wxpt[128:256], wh.T, wout.T, -wout.T, -wh.T],
        axis=1).astype(NPBF16)
    bst = np.stack([b0, b1], axis=1).astype(np.float32)        # [H, 2]

    csel = coeffs[:, idx].reshape(B, K4)                      # [B, 256]
    dsel = dcoeffs[:, idx].reshape(B, K4)
    cselT = csel.T.astype(NPBF16)                             # [256, B]
    dselT = dsel.T.astype(NPBF16)
    hT = h.T.astype(NPBF16)                                   # [128, B]

    in_maps = []
    for c in range(N_CORES):
        sl = slice(c * BS, (c + 1) * BS)
        blocks = []
        for ch in range(NCH):
            s2 = slice(c * BS + ch * CHUNK, c * BS + (ch + 1) * CHUNK)
            blocks += [cselT[0:128, s2], cselT[128:256, s2], hT[:, s2],
                       dselT[0:128, s2], dselT[128:256, s2]]
        xint = np.ascontiguousarray(np.concatenate(blocks, axis=1))
        in_maps.append({"xint": xint, "wtst": wts, "bst": bst})
    return in_maps


def kernel(**inputs) -> np.ndarray:
    in_maps = _prep_in_maps(**inputs)
    nc = _get_nc()
    res = bass_utils.run_bass_kernel_spmd(nc, in_maps,
                                          core_ids=list(range(N_CORES)))
    out = np.empty((B, H), np.float32)
    for c in range(N_CORES):
        out[c * BS:(c + 1) * BS] = res.results[c]["outt"].astype(np.float32).T
    return out


# revision 11
# speedup vs baseline: 1.0992x; 1.0057x over previous
"""Trainium2 Bass kernel for the Neural-CDE-style cell (nn_JaCDE_88167088653055).

Math (per batch row b):
    x    = spline(coeffs, t)   xdot = spline(dcoeffs, t)
    l1   = x @ wx.T + h @ wh.T + b0
    relu = relu(l1);  drelu = sigmoid(l1)
    lout = relu @ wout.T + b1; th = tanh(lout); dth = 1 - th^2
    J(v) = dth * ((drelu * v) @ wout.T)        # action of the Jacobian factor
    jx   = J(xdot @ wx.T); jxh = J(jx @ wh.T); jxhh = J(jxh @ wh.T)
    out  = jx + jxh + jxhh

Device-side structure (per core, batch-feature-major [H, batch] tiles):
  * spline folds into the wx matmul: x @ wx.T == csel_flat @ (wx (x) powers).T,
    so the contraction is K=256 and the spline costs no device passes.
  * dth = 1-tanh^2 computed as ACT Tanh then ACT Square (thq); the m-matmuls
    use a negated stationary -wout so every dth-multiply is a single
    scalar_tensor_tensor (thq-1)*M = dth*(wout@p).
  * m1+m2+m3 accumulate in ONE PSUM bank via PE start/stop flags; jx = dth*M1,
    t2 = dth*M2 (= jx+jxh), out = dth*M3 directly.  jxh is never formed:
    the g bank holds -Wh@jx, and accumulating +Wh@t2 onto it yields
    Wh@jxh directly (the sign flip is absorbed by using +wout for m2).
  * the two 512-column chunks are software-pipelined stage-by-stage so every
    engine queue alternates A/B work (in-order queues never head-of-line block
    a ready chunk behind a stalled one).
  * warmup: a dummy Sigmoid first (loads the single act table covering
    Relu+Sigmoid during the input DMA) and dummy matmuls on a zeroed tile to
    ramp the PE clock out of its low p-state before real work arrives.

Sharding: pure data parallel - batch 8192 split as 1024 rows per core across
8 cores; small weights replicated; no cross-core communication.
"""

import numpy as np
import ml_dtypes

import concourse.bass as bass
import concourse.mybir as mybir
import concourse.tile as tile
from concourse import bacc, bass_utils

N_CORES = 8
B = 8192
NOBS = 16
CIN = 64
H = 128
K4 = CIN * 4            # 256: folded (channel, power) contraction dim
BS = B // N_CORES       # 1024 batch rows per core
CHUNK = 512             # batch columns per pipeline step (one PSUM bank)
NCH = BS // CHUNK       # 2
F32 = mybir.dt.float32
BF16 = mybir.dt.bfloat16
NPBF16 = ml_dtypes.bfloat16

DUMMY_MM = 6            # PE-clock warmup matmuls (N=256 each) on zeroed data

_NC_CACHE = {}


def _build_nc():
    AF = mybir.ActivationFunctionType
    OP = mybir.AluOpType

    nc = bacc.Bacc("TRN2", target_bir_lowering=False, debug=False,
                   enable_asserts=False, num_devices=N_CORES)

    # inputs packed per chunk as [c0|c1|d0|d1|h], 5*CHUNK bf16 cols per chunk
    xint = nc.dram_tensor("xint", [128, NCH * 5 * CHUNK], BF16,
                          kind="ExternalInput")
    # weights packed [wxp0|wxp1|wht|woutt|-woutt|-wht]
    wtst = nc.dram_tensor("wtst", [128, 6 * H], BF16, kind="ExternalInput")
    bst = nc.dram_tensor("bst", [H, 2], F32, kind="ExternalInput")
    outt = nc.dram_tensor("outt", [H, BS], BF16, kind="ExternalOutput")

    def mm(out_ap, lhsT, rhs, start=True, stop=True):
        nc.tensor.matmul(out_ap, lhsT, rhs, start=start, stop=stop,
                         skip_group_check=True)

    with tile.TileContext(nc) as tc:
        with tc.tile_pool(name="sb", bufs=2) as sb, \
             tc.tile_pool(name="ps", bufs=1, space="PSUM") as ps:
            wp = io = tmp = sb

            # ---- warmup + weight/bias/input loads (t=0) ----
            dmy = wp.tile([128, CHUNK], BF16, tag="dmy")
            nc.vector.memset(dmy[:], 0.0)

            wxp0t = wp.tile([128, H], BF16, tag="wxp0t")
            nc.sync.dma_start(wxp0t[:], wtst[:, 0:H])
            wxp1t = wp.tile([128, H], BF16, tag="wxp1t")
            nc.scalar.dma_start(wxp1t[:], wtst[:, H:2 * H])
            wrest = wp.tile([128, 4 * H], BF16, tag="wrest")
            nc.gpsimd.dma_start(wrest[:], wtst[:, 2 * H:6 * H])
            bs = wp.tile([H, 2], F32, tag="bs")

            xin = []
            for ch in range(NCH):
                xt = io.tile([128, 5 * CHUNK], BF16, tag="xin")
                xin.append(xt)

            def ld(ch, blk, eng):
                lo_, hi_ = blk * CHUNK, (blk + 1) * CHUNK
                base = ch * 5 * CHUNK
                eng.dma_start(xin[ch][:, lo_:hi_], xint[:, base + lo_:base + hi_])

            # priority order: chunk A's l1 operands + weights stream first
            ld(0, 0, nc.sync)    # c0A
            ld(0, 1, nc.scalar)  # c1A
            ld(0, 2, nc.gpsimd)  # hA
            ld(0, 3, nc.sync)    # d0A
            ld(0, 4, nc.scalar)  # d1A
            nc.gpsimd.dma_start(bs[:], bst[:])
            ld(1, 0, nc.sync)    # c0B
            ld(1, 1, nc.scalar)  # c1B
            ld(1, 2, nc.gpsimd)  # hB
            ld(1, 3, nc.sync)    # d0B
            ld(1, 4, nc.scalar)  # d1B

            # act-table warmup: Sigmoid first -> single table set load that
            # also covers Relu, overlapped with the input DMA
            wrm = tmp.tile([128, 1], F32, tag="wrm")
            nc.scalar.activation(wrm[:], dmy[:, 0:1], AF.Sigmoid)

            wxp0 = wxp0t[:]
            wxp1 = wxp1t[:]
            wht = wrest[:, 0 * H:1 * H]
            wot = wrest[:, 1 * H:2 * H]
            won = wrest[:, 2 * H:3 * H]   # -wout
            whn = wrest[:, 3 * H:4 * H]   # -wh
            b0ap = bs[:, 0:1]
            b1ap = bs[:, 1:2]

            # PE-clock warmup: dummy matmuls on zeroed data into the lo bank
            lo_warm = ps.tile([H, CHUNK], F32, tag="lo")
            for _ in range(DUMMY_MM):
                mm(lo_warm[:, 0:256], dmy[:, 0:H], dmy[:, 0:256])

            # ---- software-pipelined chunk stages ----
            l1 = [None] * NCH
            u = [None] * NCH
            relu = [None] * NCH
            drelu = [None] * NCH
            lo = [None] * NCH
            q = [None] * NCH
            m = [None] * NCH
            g = [None] * NCH
            jx = [None] * NCH

            # F: front-end matmuls + relu/drelu
            for ch in range(NCH):
                xt = xin[ch]
                c0 = xt[:, 0 * CHUNK:1 * CHUNK]
                c1 = xt[:, 1 * CHUNK:2 * CHUNK]
                ht = xt[:, 2 * CHUNK:3 * CHUNK]
                d0 = xt[:, 3 * CHUNK:4 * CHUNK]
                d1 = xt[:, 4 * CHUNK:5 * CHUNK]
                l1[ch] = ps.tile([H, CHUNK], F32, tag="l1", bufs=1, name="l1")
                u[ch] = ps.tile([H, CHUNK], F32, tag="u", bufs=2, name="u")
                mm(l1[ch][:], wxp0, c0, start=True, stop=False)
                mm(l1[ch][:], wxp1, c1, start=False, stop=False)
                mm(l1[ch][:], wht, ht, start=False, stop=True)
                mm(u[ch][:], wxp0, d0, start=True, stop=False)
                mm(u[ch][:], wxp1, d1, start=False, stop=True)
                relu[ch] = tmp.tile([H, CHUNK], BF16, tag="relu", name="relu")
                nc.scalar.activation(relu[ch][:], l1[ch][:], AF.Relu,
                                     bias=b0ap)
                drelu[ch] = tmp.tile([H, CHUNK], F32, tag="drelu", name="drelu")
                nc.scalar.activation(drelu[ch][:], l1[ch][:], AF.Sigmoid,
                                     bias=b0ap)

            # C1: lout, s, q, p1, m1
            for ch in range(NCH):
                lo[ch] = ps.tile([H, CHUNK], F32, tag="lo", name="lo")
                mm(lo[ch][:], wot, relu[ch][:])
                th = tmp.tile([H, CHUNK], F32, tag="th")
                nc.scalar.activation(th[:], lo[ch][:], AF.Tanh, bias=b1ap)
                q[ch] = tmp.tile([H, CHUNK], F32, tag="q", name="q")
                nc.scalar.activation(q[ch][:], th[:], AF.Square)
                p1 = tmp.tile([H, CHUNK], BF16, tag="p1")
                nc.vector.tensor_mul(p1[:], drelu[ch][:], u[ch][:])
                m[ch] = ps.tile([H, CHUNK], F32, tag="m", bufs=2, name="m")
                mm(m[ch][:], won, p1[:], start=True, stop=False)

            # C2: jx = dth*M1, g1, p2, m2  (dth = -4q; 4 folded into wo4)
            for ch in range(NCH):
                jx[ch] = tmp.tile([H, CHUNK], BF16, tag="jx", name="jx")
                nc.vector.scalar_tensor_tensor(jx[ch][:], q[ch][:], 1.0,
                                               m[ch][:], OP.subtract, OP.mult)
                g[ch] = ps.tile([H, CHUNK], F32, tag="g", bufs=2, name="g")
                mm(g[ch][:], whn, jx[ch][:], start=True, stop=False)
                p2 = tmp.tile([H, CHUNK], BF16, tag="p2")
                nc.vector.tensor_mul(p2[:], drelu[ch][:], g[ch][:])
                mm(m[ch][:], wot, p2[:], start=False, stop=False)

            # C3: t2 = dth*M2 = jx+jxh; accumulate Wh@t2 onto g (= -Wh@jx)
            # so the bank becomes Wh@jxh with no explicit subtract; then
            # p3, m3, and the final out = dth*M3 + its DMA per chunk.
            # The last chunk runs p3/m3/outs at half width so its output
            # DMA starts earlier (the absolute tail of the kernel).
            for ch in range(NCH):
                t2 = tmp.tile([H, CHUNK], BF16, tag="t2")
                nc.vector.scalar_tensor_tensor(t2[:], q[ch][:], 1.0,
                                               m[ch][:], OP.subtract, OP.mult)
                mm(g[ch][:], wht, t2[:], start=False, stop=True)  # = Wh@jxh
                p3 = tmp.tile([H, CHUNK], BF16, tag="p3")
                outs = tmp.tile([H, CHUNK], BF16, tag="outs")
                base = ch * CHUNK
                if ch < NCH - 1:
                    nc.vector.tensor_mul(p3[:], drelu[ch][:], g[ch][:])
                    mm(m[ch][:], won, p3[:], start=False, stop=True)
                    nc.vector.scalar_tensor_tensor(outs[:], q[ch][:], 1.0,
                                                   m[ch][:], OP.subtract,
                                                   OP.mult)
                    nc.gpsimd.dma_start(outt[:, base:base + CHUNK], outs[:])
                else:
                    hw_ = CHUNK // 2
                    for hf in range(2):
                        sl = slice(hf * hw_, (hf + 1) * hw_)
                        nc.vector.tensor_mul(p3[:, sl], drelu[ch][:, sl],
                                             g[ch][:, sl])
                        mm(m[ch][:, sl], won, p3[:, sl], start=False,
                           stop=(hf == 1))
                        nc.vector.scalar_tensor_tensor(
                            outs[:, sl], q[ch][:, sl], 1.0, m[ch][:, sl],
                            OP.subtract, OP.mult)
                        nc.gpsimd.dma_start(
                            outt[:, base + hf * hw_:base + (hf + 1) * hw_],
                            outs[:, sl])

    nc.compile()
    return nc


def _get_nc():
    if "nc" not in _NC_CACHE:
        _NC_CACHE["nc"] = _build_nc()
    return _NC_CACHE["nc"]


def _prep_in_maps(t, h, coeffs, dcoeffs, tobs, wx, wh, wout, b0, b1):
    t = np.asarray(t, np.float32)
    h = np.asarray(h, np.float32)
    coeffs = np.asarray(coeffs, np.float32)
    dcoeffs = np.asarray(dcoeffs, np.float32)
    tobs = np.asarray(tobs, np.float32)
    wx = np.asarray(wx, np.float32)
    wh = np.asarray(wh, np.float32)
    wout = np.asarray(wout, np.float32)
    b0 = np.asarray(b0, np.float32)
    b1 = np.asarray(b1, np.float32)

    ts = t[0]
    idx = int(np.clip(np.searchsorted(tobs, ts, side="right") - 1, 0, NOBS - 2))
    dtv = np.float32(ts - tobs[idx])
    powers = dtv ** np.arange(4, dtype=np.float32)            # [4]
    wxp = (wx[:, :, None] * powers[None, None, :]).reshape(H, K4)

    wxpt = wxp.T                                              # [256, 128]
    # packed stationaries: [wxp0|wxp1|wht|woutt|4*woutt], bf16
    wts = np.concatenate(
        [wxpt[0:128], wxpt[128:256], wh.T, wout.T, -wout.T, -wh.T],
        axis=1).astype(NPBF16)
    bst = np.stack([b0, b1], axis=1).astype(np.float32)        # [H, 2]

    csel = coeffs[:, idx].reshape(B, K4)                      # [B, 256]
    dsel = dcoeffs[:, idx].reshape(B, K4)
    cselT = csel.T.astype(NPBF16)                             # [256, B]
    dselT = dsel.T.astype(NPBF16)
    hT = h.T.astype(NPBF16)                                   # [128, B]

    in_maps = []
    for c in range(N_CORES):
        sl = slice(c * BS, (c + 1) * BS)
        blocks = []
        for ch in range(NCH):
            s2 = slice(c * BS + ch * CHUNK, c * BS + (ch + 1) * CHUNK)
            blocks += [cselT[0:128, s2], cselT[128:256, s2], hT[:, s2],
                       dselT[0:128, s2], dselT[128:256, s2]]
        xint = np.ascontiguousarray(np.concatenate(blocks, axis=1))
        in_maps.append({"xint": xint, "wtst": wts, "bst": bst})
    return in_maps


def kernel(**inputs) -> np.ndarray:
    in_maps = _prep_in_maps(**inputs)
    nc = _get_nc()
    res = bass_utils.run_bass_kernel_spmd(nc, in_maps,
                                          core_ids=list(range(N_CORES)))
    out = np.empty((B, H), np.float32)
    for c in range(N_CORES):
        out[c * BS:(c + 1) * BS] = res.results[c]["outt"].astype(np.float32).T
    return out
